# revision 1
# baseline (speedup 1.0000x reference)
"""Trainium2 Bass kernel: per-pixel 19x19 batch blur (KPN-style).

Reference computation:
    out[b,c,i,j] = (1/361) * sum_{ki,kj} pad[b,c,i+ki,j+kj] * kern[b, i*W+j, ki, kj]
with `pad` the 9-pixel reflection-padded input, shapes:
    input  (2, 3, 256, 256) f32
    kernel (2, 65536, 19, 19) f32    <- 189 MB, dominates memory traffic

Sharding: pure data parallel over (batch, H-tile): 8 cores = 2 batches x 4
tiles of 64 output rows each.  Each core receives
  - its contiguous kernel slice  (16384, 361) f32  (23.7 MB)
  - SHIFTC [2, 3, 128, 19*82] f32 (4.8 MB): for jblk/channel, partition j
    holds the flattened sliding strips  SHIFTC[jb,c,j, 19*r+kj] =
    pad[c, r, jb*128+j+kj].  This is the im2col halo prep done host-side
    (19x replication of the 0.27 MB padded slice); with this layout the
    361-tap patch of output row i is CONTIGUOUS at free offset 19*i,
    because k2 = 19*ki + kj.
and writes a (256, 192) f32 block = [jblk*128+j, c*64+i] that the host
transposes back into (3, 64, 256).

Device algorithm (per core), pixels-on-partitions:
  - kernel slice streamed in chunks [128 parts (j) x (R_CHUNK rows * 361)]
  - per (row i, jblk, channel c): ONE fused VectorE scalar_tensor_tensor:
        out  = (kern_row * 1/361) * patch_view      (361 contiguous f32)
        accum_out[j] = sum(out)                     -> output column
    (the ant-custom tensor_tensor_reduce faults on this runtime;
     InstTensorScalarPtr with accum output is standard ISA and works)
"""

import os
import sys

import numpy as np

for _p in ("/opt/trn_rl_repo", os.path.expanduser("~/.axon_site/_ro/trn_rl_repo")):
    if os.path.isdir(_p) and _p not in sys.path:
        sys.path.insert(0, _p)

from contextlib import ExitStack

from concourse import bacc, bass_utils, mybir, tile
from concourse.ap import AP

# Problem constants (hardcoded per the self-containment contract).
B, C, H, W = 2, 3, 256, 256
L = 19
PAD = L // 2  # 9
K2 = L * L  # 361
N_CORES = 8
ROWS_PER_CORE = H // 4  # 64  (4 H-tiles x 2 batches = 8 cores)
PR = ROWS_PER_CORE + 2 * PAD  # 82 padded rows per core
SFREE = L * PR  # 1558 free elems per SHIFTC partition
R_CHUNK = 8  # output rows per kernel-DMA chunk
N_IBLK = ROWS_PER_CORE // R_CHUNK  # 8
F32 = mybir.dt.float32

_CACHE: dict = {}


def _build_program():
    nc = bacc.Bacc(
        "TRN2",
        target_bir_lowering=False,
        debug=False,
        enable_asserts=False,
        num_devices=N_CORES,
    )
    kern = nc.dram_tensor("kern", [ROWS_PER_CORE * W, K2], F32, kind="ExternalInput")
    shiftd = nc.dram_tensor("shiftc", [2, C, 128, SFREE], F32, kind="ExternalInput")
    outd = nc.dram_tensor("out", [2 * 128, C * ROWS_PER_CORE], F32, kind="ExternalOutput")

    mult = mybir.AluOpType.mult

    with tile.TileContext(nc) as tc, ExitStack() as ctx:
        cpool = ctx.enter_context(tc.tile_pool(name="cpool", bufs=1))
        kpool = ctx.enter_context(tc.tile_pool(name="kpool", bufs=3))
        spool = ctx.enter_context(tc.tile_pool(name="spool", bufs=2))

        # SHIFTC tiles: issue the first-needed one, then the first kernel
        # chunk, then the rest — so the first STT's inputs aren't queued
        # behind 4.8 MB of later SHIFTC traffic.
        shiftc = {}

        def _load_shiftc(c, jb):
            sc = cpool.tile([128, SFREE], F32, name=f"shiftc_{c}_{jb}")
            shiftc[(c, jb)] = sc
            nc.sync.dma_start(
                out=sc[:, :],
                in_=AP(shiftd, (jb * C + c) * 128 * SFREE, [(SFREE, 128), (1, SFREE)]),
            )

        kfree = R_CHUNK * K2

        def _load_chunk(iblk, jb):
            kt = kpool.tile([128, kfree], F32, name="kt", tag="kt")
            base = (iblk * R_CHUNK * W + jb * 128) * K2
            nc.sync.dma_start(
                out=AP(kt.tensor, 0, [(kfree, 128), (K2, R_CHUNK), (1, K2)]),
                in_=AP(kern, base, [(K2, 128), (W * K2, R_CHUNK), (1, K2)]),
            )
            return kt

        _load_shiftc(0, 0)
        kts = {(0, 0): _load_chunk(0, 0)}
        for c in range(1, C):
            _load_shiftc(c, 0)
        for c in range(C):
            _load_shiftc(c, 1)

        outt = [cpool.tile([128, C * ROWS_PER_CORE], F32, name=f"outt{jb}") for jb in range(2)]

        for iblk in range(N_IBLK):
            for jb in range(2):
                kt = kts.pop((iblk, jb), None)
                if kt is None:
                    kt = _load_chunk(iblk, jb)
                for c in range(C):
                    for il in range(R_CHUNK):
                        i = iblk * R_CHUNK + il
                        scr = spool.tile([128, K2], F32, name="scr", tag="scr")
                        col = c * ROWS_PER_CORE + i
                        # out = (in0 * 1/361) * in1 ; accum_out = sum(out).
                        # All three APs are flat contiguous 361-elem runs.
                        nc.vector.scalar_tensor_tensor(
                            out=scr[:, :],
                            in0=AP(kt.tensor, il * K2, [(kfree, 128), (1, K2)]),
                            scalar=1.0 / K2,
                            in1=AP(shiftc[(c, jb)].tensor, i * L, [(SFREE, 128), (1, K2)]),
                            op0=mult,
                            op1=mult,
                            accum_out=outt[jb][:, col : col + 1],
                        )

        for jb in range(2):
            nc.sync.dma_start(
                out=AP(
                    outd,
                    jb * 128 * (C * ROWS_PER_CORE),
                    [(C * ROWS_PER_CORE, 128), (1, C * ROWS_PER_CORE)],
                ),
                in_=outt[jb][:, :],
            )

    nc.compile()
    return nc


def _program():
    if "nc" not in _CACHE:
        _CACHE["nc"] = _build_program()
    return _CACHE["nc"]


def _shard_inputs(input, kernel):
    inp = np.ascontiguousarray(np.asarray(input, dtype=np.float32))
    kern = np.asarray(kernel, dtype=np.float32)
    pad = np.pad(inp, ((0, 0), (0, 0), (PAD, PAD), (PAD, PAD)), mode="reflect")
    # sliding horizontal strips: strips[b, c, r, j, kj] = pad[b, c, r, j + kj]
    strips = np.lib.stride_tricks.sliding_window_view(pad, L, axis=3)
    in_maps = []
    for core in range(N_CORES):
        b, q = divmod(core, 4)
        r0 = q * ROWS_PER_CORE
        # SHIFTC[jb, c, j, 19*r + kj] = pad[b, c, r0 + r, jb*128 + j + kj]
        s = strips[b, :, r0 : r0 + PR, :, :]  # (C, PR, 256, L)
        s = s.transpose(2, 0, 1, 3).reshape(2, 128, C, PR * L)  # (jb*128+j, c, r*L+kj)
        sc = np.ascontiguousarray(s.transpose(0, 2, 1, 3))  # (2, C, 128, SFREE)
        ks = np.ascontiguousarray(
            kern[b].reshape(H * W, K2)[q * ROWS_PER_CORE * W : (q + 1) * ROWS_PER_CORE * W]
        )
        in_maps.append({"kern": ks, "shiftc": sc})
    return in_maps


def _unshard_output(results):
    out = np.empty((B, C, H, W), dtype=np.float32)
    for core in range(N_CORES):
        b, q = divmod(core, 4)
        arr = np.asarray(results[core]["out"])  # (256, 192) = [jb*128+j, c*64+i]
        blk = arr.reshape(2, 128, C, ROWS_PER_CORE).transpose(2, 3, 0, 1)
        out[b, :, q * ROWS_PER_CORE : (q + 1) * ROWS_PER_CORE, :] = blk.reshape(
            C, ROWS_PER_CORE, W
        )
    return out


def run_sharded(inputs, **kw):
    """Run the compiled SPMD program; returns BassKernelResults (for profiling)."""
    in_maps = _shard_inputs(inputs["input"], inputs["kernel"])
    return bass_utils.run_bass_kernel_spmd(
        _program(), in_maps, core_ids=list(range(N_CORES)), **kw
    )


def kernel(input, kernel):
    res = run_sharded({"input": input, "kernel": kernel})
    return _unshard_output(res.results)



# revision 2
# speedup vs baseline: 1.0392x; 1.0392x over previous
"""Trainium2 Bass kernel: per-pixel 19x19 batch blur (KPN-style).

Reference computation:
    out[b,c,i,j] = (1/361) * sum_{ki,kj} pad[b,c,i+ki,j+kj] * kern[b, i*W+j, ki, kj]
with `pad` the 9-pixel reflection-padded input, shapes:
    input  (2, 3, 256, 256) f32
    kernel (2, 65536, 19, 19) f32    <- dominates memory traffic

Sharding: pure data parallel over (batch, H-tile): 8 cores = 2 batches x 4
tiles of 64 output rows each.

fp16 design (vs the f32 baseline at ~198us):
  The kernel is DVE-bound: 384 scalar_tensor_tensor instrs/core (one per
  (row i, jblk, channel)), each a 361-tap multiply+accumulate over 128
  pixel-partitions.  fp32 TT-class ops run at 1x (per-instr ~(FD+151)/0.96
  ns); 16-bit operands with step-1, 4B-aligned APs engage 2x_1P mode.
  To make EVERY row's patch window 4B-aligned we use a stride-20 tap
  layout: k20 = 20*ki + kj with the kj=19 slot zeroed in the weights, so
  the 380-elem window of output row i starts at free offset 20*i (40*i
  bytes).  Host-side prep:
  - kern slice packed to the exact SBUF chunk layout [16, 128, 8*380] f16
    (chunk = (iblk, jb); partition j; il-major taps), zero at kj=19 -> each
    chunk DMA is one contiguous [128, 6080B] block.
  - SHIFTC20 [2, C, 128, 82*20] f16: partition j holds sliding strips
    SHIFTC20[jb,c,j, 20*r+kj] = pad[c, r, jb*128+j+kj] (kj slot 19 holds a
    harmless real/pad value; it is multiplied by the zeroed weight slot).
  Per-instr STT: out_f16[128,380] = (kern_row * 1/361) * patch_window,
  accum_out (f32) = per-partition sum -> one output column.
"""

import os
import sys

import numpy as np

for _p in ("/opt/trn_rl_repo", os.path.expanduser("~/.axon_site/_ro/trn_rl_repo")):
    if os.path.isdir(_p) and _p not in sys.path:
        sys.path.insert(0, _p)

from contextlib import ExitStack

from concourse import bacc, bass_utils, mybir, tile
from concourse.ap import AP

# Problem constants (hardcoded per the self-containment contract).
B, C, H, W = 2, 3, 256, 256
L = 19
L20 = 20  # padded tap-row stride (kj slot 19 zeroed in weights)
PAD = L // 2  # 9
K2 = L * L  # 361
K20 = L * L20  # 380 taps per instr incl. zero slots
N_CORES = 8
ROWS_PER_CORE = H // 4  # 64  (4 H-tiles x 2 batches = 8 cores)
PR = ROWS_PER_CORE + 2 * PAD  # 82 padded rows per core
SFREE = L20 * PR  # 1640 free elems per SHIFTC partition
R_CHUNK = 8  # output rows per kernel-DMA chunk
N_IBLK = ROWS_PER_CORE // R_CHUNK  # 8
F32 = mybir.dt.float32
F16 = mybir.dt.float16

_CACHE: dict = {}


def _build_program():
    nc = bacc.Bacc(
        "TRN2",
        target_bir_lowering=False,
        debug=False,
        enable_asserts=False,
        num_devices=N_CORES,
    )
    kfree = R_CHUNK * K20  # 3040
    kern = nc.dram_tensor("kern", [2 * N_IBLK * 128, kfree], F16, kind="ExternalInput")
    shiftd = nc.dram_tensor("shiftc", [2, C, 128, SFREE], F16, kind="ExternalInput")
    outd = nc.dram_tensor("out", [2 * 128, C * ROWS_PER_CORE], F32, kind="ExternalOutput")

    mult = mybir.AluOpType.mult

    with tile.TileContext(nc) as tc, ExitStack() as ctx:
        cpool = ctx.enter_context(tc.tile_pool(name="cpool", bufs=1))
        kpool = ctx.enter_context(tc.tile_pool(name="kpool", bufs=3))
        spool = ctx.enter_context(tc.tile_pool(name="spool", bufs=2))

        # SHIFTC tiles: issue the first-needed one, then the first kernel
        # chunk, then the rest — so the first STT's inputs aren't queued
        # behind the later SHIFTC traffic.
        shiftc = {}

        def _load_shiftc(c, jb):
            sc = cpool.tile([128, SFREE], F16, name=f"shiftc_{c}_{jb}")
            shiftc[(c, jb)] = sc
            nc.sync.dma_start(
                out=sc[:, :],
                in_=AP(shiftd, (jb * C + c) * 128 * SFREE, [(SFREE, 128), (1, SFREE)]),
            )

        def _load_chunk(iblk, jb):
            kt = kpool.tile([128, kfree], F16, name="kt", tag="kt")
            base = (iblk * 2 + jb) * 128 * kfree
            nc.sync.dma_start(
                out=kt[:, :],
                in_=AP(kern, base, [(kfree, 128), (1, kfree)]),
            )
            return kt

        _load_shiftc(0, 0)
        kts = {(0, 0): _load_chunk(0, 0)}
        for c in range(1, C):
            _load_shiftc(c, 0)
        for c in range(C):
            _load_shiftc(c, 1)

        outt = [cpool.tile([128, C * ROWS_PER_CORE], F32, name=f"outt{jb}") for jb in range(2)]

        for iblk in range(N_IBLK):
            for jb in range(2):
                kt = kts.pop((iblk, jb), None)
                if kt is None:
                    kt = _load_chunk(iblk, jb)
                for c in range(C):
                    for il in range(R_CHUNK):
                        i = iblk * R_CHUNK + il
                        scr = spool.tile([128, K20], F16, name="scr", tag="scr")
                        col = c * ROWS_PER_CORE + i
                        # out = (in0 * 1/361) * in1 ; accum_out = sum(out).
                        # All APs are flat contiguous 380-elem f16 runs at
                        # 4B-aligned offsets -> DVE 2x_1P mode.
                        nc.vector.scalar_tensor_tensor(
                            out=scr[:, :],
                            in0=AP(kt.tensor, il * K20, [(kfree, 128), (1, K20)]),
                            scalar=1.0 / K2,
                            in1=AP(shiftc[(c, jb)].tensor, i * L20, [(SFREE, 128), (1, K20)]),
                            op0=mult,
                            op1=mult,
                            accum_out=outt[jb][:, col : col + 1],
                        )

        for jb in range(2):
            nc.sync.dma_start(
                out=AP(
                    outd,
                    jb * 128 * (C * ROWS_PER_CORE),
                    [(C * ROWS_PER_CORE, 128), (1, C * ROWS_PER_CORE)],
                ),
                in_=outt[jb][:, :],
            )

    nc.compile()
    return nc


def _program():
    if "nc" not in _CACHE:
        _CACHE["nc"] = _build_program()
    return _CACHE["nc"]


def _shard_inputs(input, kernel):
    inp = np.ascontiguousarray(np.asarray(input, dtype=np.float32))
    kern16 = np.asarray(kernel, dtype=np.float32).astype(np.float16)

    # kern: (B, 65536, 19, 19) f16 -> per-core chunk layout
    #   [b, q, chunk=(iblk,jb), j, il, ki, kj20] with kj slot 19 zeroed.
    kr = kern16.reshape(B, 4, N_IBLK, R_CHUNK, 2, 128, L, L)  # b q iblk il jb j ki kj
    kr = np.pad(kr, ((0, 0),) * 7 + ((0, 1),))  # kj -> 20, zero slot
    kr = kr.transpose(0, 1, 2, 4, 5, 3, 6, 7)  # b q iblk jb j il ki kj20
    kr = np.ascontiguousarray(kr).reshape(B, 4, 2 * N_IBLK * 128, R_CHUNK * K20)

    pad = np.pad(inp, ((0, 0), (0, 0), (PAD, PAD), (PAD, PAD)), mode="reflect")
    pad = np.pad(pad, ((0, 0), (0, 0), (0, 0), (0, 1)))  # extra col so window-20 fits
    pad16 = pad.astype(np.float16)
    # sliding horizontal strips: strips[b, c, r, j, kj20] = pad[b, c, r, j + kj]
    strips = np.lib.stride_tricks.sliding_window_view(pad16, L20, axis=3)
    in_maps = []
    for core in range(N_CORES):
        b, q = divmod(core, 4)
        r0 = q * ROWS_PER_CORE
        # SHIFTC20[jb, c, j, 20*r + kj] = pad[b, c, r0 + r, jb*128 + j + kj]
        s = strips[b, :, r0 : r0 + PR, :, :]  # (C, PR, 256, 20)
        s = s.transpose(2, 0, 1, 3).reshape(2, 128, C, PR * L20)  # (jb*128+j, c, r*20+kj)
        sc = np.ascontiguousarray(s.transpose(0, 2, 1, 3))  # (2, C, 128, SFREE)
        in_maps.append({"kern": kr[b, q], "shiftc": sc})
    return in_maps


def _unshard_output(results):
    out = np.empty((B, C, H, W), dtype=np.float32)
    for core in range(N_CORES):
        b, q = divmod(core, 4)
        arr = np.asarray(results[core]["out"])  # (256, 192) = [jb*128+j, c*64+i]
        blk = arr.reshape(2, 128, C, ROWS_PER_CORE).transpose(2, 3, 0, 1)
        out[b, :, q * ROWS_PER_CORE : (q + 1) * ROWS_PER_CORE, :] = blk.reshape(
            C, ROWS_PER_CORE, W
        )
    return out


def run_sharded(inputs, **kw):
    """Run the compiled SPMD program; returns BassKernelResults (for profiling)."""
    in_maps = _shard_inputs(inputs["input"], inputs["kernel"])
    return bass_utils.run_bass_kernel_spmd(
        _program(), in_maps, core_ids=list(range(N_CORES)), **kw
    )


def kernel(input, kernel):
    res = run_sharded({"input": input, "kernel": kernel})
    return _unshard_output(res.results)


# revision 4
# speedup vs baseline: 1.0792x; 1.0385x over previous
"""Trainium2 Bass kernel: per-pixel 19x19 batch blur (KPN-style).

Reference computation:
    out[b,c,i,j] = (1/361) * sum_{ki,kj} pad[b,c,i+ki,j+kj] * kern[b, i*W+j, ki, kj]
with `pad` the 9-pixel reflection-padded input, shapes:
    input  (2, 3, 256, 256) f32
    kernel (2, 65536, 19, 19) f32    <- dominates memory traffic

Sharding: pure data parallel over (batch, H-tile): 8 cores = 2 batches x 4
tiles of 64 output rows each.

Hybrid DVE+ScalarE design (vs the f32 STT-only baseline at ~198us):
  The kernel is bound by per-pixel 361-tap dot products: 384 row-instrs/core
  (one per (row i, jblk, channel) over 128 pixel-partitions).  Measured facts
  on this silicon:
    - scalar_tensor_tensor (fused mul+accum) runs at 1x only: ~456ns/row.
    - plain tensor_tensor in fp16 hits the 2x_1P perf mode (~200ns/row when
      row-batched), but has no accumulator.
    - ScalarE activation(Copy, accum_out) reduces a 362-elem row in ~490ns,
      CONCURRENTLY with the DVE.
  So per 24-row block we split: 9 rows stay fused-STT on DVE; 15 rows are
  premultiplied on DVE via 2x tensor_tensor (channel-broadcast, row-batched)
  and reduced on ScalarE.  Both engines run ~7.3us/block in parallel.

  Layout: kern rows padded 361->362 (zero slot); patches as sliding strips
  SHIFTC[j, 19*r + kj] = pad[r, j+kj] in TWO copies: S1 shifted one element
  right, so odd output rows read at even (4B-aligned) offsets 19i+1 and
  even rows at 19i from S0 -- keeping every tensor_tensor operand run
  4B-aligned for the 2x mode.  fp16 operands, fp32 accumulation.
"""

import os
import sys

import numpy as np

for _p in ("/opt/trn_rl_repo", os.path.expanduser("~/.axon_site/_ro/trn_rl_repo")):
    if os.path.isdir(_p) and _p not in sys.path:
        sys.path.insert(0, _p)

from contextlib import ExitStack

from concourse import bacc, bass_utils, mybir, tile
from concourse.ap import AP

# Problem constants (hardcoded per the self-containment contract).
B, C, H, W = 2, 3, 256, 256
L = 19
PAD = L // 2  # 9
K2 = L * L  # 361
KP = K2 + 1  # 362: padded row length (slot 361 zeroed)
N_CORES = 8
ROWS_PER_CORE = H // 4  # 64  (4 H-tiles x 2 batches = 8 cores)
PR = ROWS_PER_CORE + 2 * PAD  # 82 padded rows per core
SF1 = L * PR + 2  # 1560 free elems per SHIFTC channel (2 pad)
R_CHUNK = 8  # output rows per kernel-DMA chunk
N_IBLK = ROWS_PER_CORE // R_CHUNK  # 8
KFREE = R_CHUNK * KP  # 2896
F32 = mybir.dt.float32
F16 = mybir.dt.float16

# Per-(block, channel) row split: B-rows premultiplied on DVE (2x TT) and
# reduced on ScalarE; A-rows fused STT on DVE.  Balanced so both engines
# run ~7.3us per 24-row block.
B_EVEN = (0, 2, 4, 6)  # read S0 at 19*i   (even offsets)
B_ODD = (1,)  # read S1 at 19*i+1 (even offsets)
A_ROWS = (3, 5, 7)  # fused STT, S1 (alignment irrelevant at 1x)

_CACHE: dict = {}


def _build_program():
    nc = bacc.Bacc(
        "TRN2",
        target_bir_lowering=False,
        debug=False,
        enable_asserts=False,
        num_devices=N_CORES,
    )
    kern = nc.dram_tensor("kern", [2 * N_IBLK * 128, KFREE], F16, kind="ExternalInput")
    # [copy(S0/S1), jb, j, c, f]
    shiftd = nc.dram_tensor("shiftc", [2, 2, 128, C * SF1], F16, kind="ExternalInput")
    outd = nc.dram_tensor("out", [2 * 128, C * ROWS_PER_CORE], F32, kind="ExternalOutput")

    mult = mybir.AluOpType.mult
    copyfn = mybir.ActivationFunctionType.Copy
    NBE = len(B_EVEN)
    NBO = len(B_ODD)

    with tile.TileContext(nc) as tc, ExitStack() as ctx:
        cpool = ctx.enter_context(tc.tile_pool(name="cpool", bufs=1))
        kpool = ctx.enter_context(tc.tile_pool(name="kpool", bufs=3))
        spool = ctx.enter_context(tc.tile_pool(name="spool", bufs=2))
        qpool = ctx.enter_context(tc.tile_pool(name="qpool", bufs=3))

        shiftc = {}

        def _load_shiftc(cp, jb):
            sc = cpool.tile([128, C * SF1], F16, name=f"shiftc_{cp}_{jb}")
            shiftc[(cp, jb)] = sc
            nc.sync.dma_start(
                out=sc[:, :],
                in_=AP(
                    shiftd,
                    (cp * 2 + jb) * 128 * C * SF1,
                    [(C * SF1, 128), (1, C * SF1)],
                ),
            )

        def _load_chunk(iblk, jb):
            kt = kpool.tile([128, KFREE], F16, name="kt", tag="kt")
            base = (iblk * 2 + jb) * 128 * KFREE
            nc.sync.dma_start(
                out=kt[:, :], in_=AP(kern, base, [(KFREE, 128), (1, KFREE)])
            )
            return kt

        _load_shiftc(0, 0)
        kts = {(0, 0): _load_chunk(0, 0)}
        _load_shiftc(1, 0)
        _load_shiftc(0, 1)
        _load_shiftc(1, 1)

        outt = [cpool.tile([128, C * ROWS_PER_CORE], F32, name=f"outt{jb}") for jb in range(2)]

        for iblk in range(N_IBLK):
            for jb in range(2):
                kt = kts.pop((iblk, jb), None)
                if kt is None:
                    kt = _load_chunk(iblk, jb)
                s0 = shiftc[(0, jb)]
                s1 = shiftc[(1, jb)]

                # --- B rows: one 2x TT per parity group, channel-broadcast ---
                qe = qpool.tile([128, C * NBE * KP], F16, name="qe", tag="qe")
                i0 = iblk * R_CHUNK + B_EVEN[0]
                nc.vector.tensor_tensor(
                    out=AP(qe.tensor, 0, [(C * NBE * KP, 128), (NBE * KP, C), (KP, NBE), (1, KP)]),
                    in0=AP(kt.tensor, B_EVEN[0] * KP, [(KFREE, 128), (0, C), (2 * KP, NBE), (1, KP)]),
                    in1=AP(s0.tensor, L * i0, [(C * SF1, 128), (SF1, C), (2 * L, NBE), (1, KP)]),
                    op=mult,
                )
                qo = qpool.tile([128, C * NBO * KP], F16, name="qo", tag="qo")
                i1 = iblk * R_CHUNK + B_ODD[0]
                nc.vector.tensor_tensor(
                    out=AP(qo.tensor, 0, [(C * NBO * KP, 128), (NBO * KP, C), (KP, NBO), (1, KP)]),
                    in0=AP(kt.tensor, B_ODD[0] * KP, [(KFREE, 128), (0, C), (2 * KP, NBO), (1, KP)]),
                    in1=AP(s1.tensor, L * i1 + 1, [(C * SF1, 128), (SF1, C), (2 * L, NBO), (1, KP)]),
                    op=mult,
                )

                # --- ScalarE reductions of the B rows ---
                for c in range(C):
                    for bi, il in enumerate(B_EVEN):
                        i = iblk * R_CHUNK + il
                        col = c * ROWS_PER_CORE + i
                        scr = spool.tile([128, KP], F16, name="ascr", tag="ascr")
                        nc.scalar.activation(
                            out=scr[:, :],
                            in_=AP(qe.tensor, (c * NBE + bi) * KP, [(C * NBE * KP, 128), (1, KP)]),
                            func=copyfn,
                            scale=1.0 / K2,
                            accum_out=outt[jb][:, col : col + 1],
                        )
                    for bi, il in enumerate(B_ODD):
                        i = iblk * R_CHUNK + il
                        col = c * ROWS_PER_CORE + i
                        scr = spool.tile([128, KP], F16, name="ascr", tag="ascr")
                        nc.scalar.activation(
                            out=scr[:, :],
                            in_=AP(qo.tensor, (c * NBO + bi) * KP, [(C * NBO * KP, 128), (1, KP)]),
                            func=copyfn,
                            scale=1.0 / K2,
                            accum_out=outt[jb][:, col : col + 1],
                        )

                # --- A rows: fused STT on DVE (1x) ---
                for c in range(C):
                    for il in A_ROWS:
                        i = iblk * R_CHUNK + il
                        col = c * ROWS_PER_CORE + i
                        scr = spool.tile([128, KP], F16, name="vscr", tag="vscr")
                        nc.vector.scalar_tensor_tensor(
                            out=scr[:, :],
                            in0=AP(kt.tensor, il * KP, [(KFREE, 128), (1, KP)]),
                            scalar=1.0 / K2,
                            in1=AP(s1.tensor, c * SF1 + L * i + 1, [(C * SF1, 128), (1, KP)]),
                            op0=mult,
                            op1=mult,
                            accum_out=outt[jb][:, col : col + 1],
                        )

        for jb in range(2):
            nc.sync.dma_start(
                out=AP(
                    outd,
                    jb * 128 * (C * ROWS_PER_CORE),
                    [(C * ROWS_PER_CORE, 128), (1, C * ROWS_PER_CORE)],
                ),
                in_=outt[jb][:, :],
            )

    nc.compile()
    return nc


def _program():
    if "nc" not in _CACHE:
        _CACHE["nc"] = _build_program()
    return _CACHE["nc"]


def _shard_inputs(input, kernel):
    inp = np.ascontiguousarray(np.asarray(input, dtype=np.float32))
    kern16 = np.asarray(kernel, dtype=np.float32).astype(np.float16)

    # kern -> per-core chunks [b, q, (iblk, jb), j, il, k(362 zero-padded)]
    kr = kern16.reshape(B, 4, N_IBLK, R_CHUNK, 2, 128, K2)  # b q iblk il jb j k
    kr = np.pad(kr, ((0, 0),) * 6 + ((0, 1),))  # k -> 362, zero slot
    kr = kr.transpose(0, 1, 2, 4, 5, 3, 6)  # b q iblk jb j il k
    kr = np.ascontiguousarray(kr).reshape(B, 4, 2 * N_IBLK * 128, KFREE)

    pad = np.pad(inp, ((0, 0), (0, 0), (PAD, PAD), (PAD, PAD)), mode="reflect")
    pad16 = pad.astype(np.float16)
    # strips[b, c, r, j, kj] = pad[b, c, r, j + kj]
    strips = np.lib.stride_tricks.sliding_window_view(pad16, L, axis=3)
    in_maps = []
    for core in range(N_CORES):
        b, q = divmod(core, 4)
        r0 = q * ROWS_PER_CORE
        s = strips[b, :, r0 : r0 + PR, :, :]  # (C, PR, 256, L)
        s = s.transpose(2, 0, 1, 3).reshape(256, C, PR * L)  # (j2, c, 19r+kj)
        s0 = np.zeros((256, C, SF1), dtype=np.float16)
        s0[:, :, : PR * L] = s
        s1 = np.zeros_like(s0)
        s1[:, :, 1:] = s0[:, :, :-1]
        # dram layout [copy, jb, j, c*SF1]
        sc = np.stack([s0, s1]).reshape(2, 2, 128, C * SF1)
        in_maps.append({"kern": kr[b, q], "shiftc": np.ascontiguousarray(sc)})
    return in_maps


def _unshard_output(results):
    out = np.empty((B, C, H, W), dtype=np.float32)
    for core in range(N_CORES):
        b, q = divmod(core, 4)
        arr = np.asarray(results[core]["out"])  # (256, 192) = [jb*128+j, c*64+i]
        blk = arr.reshape(2, 128, C, ROWS_PER_CORE).transpose(2, 3, 0, 1)
        out[b, :, q * ROWS_PER_CORE : (q + 1) * ROWS_PER_CORE, :] = blk.reshape(
            C, ROWS_PER_CORE, W
        )
    return out


def run_sharded(inputs, **kw):
    """Run the compiled SPMD program; returns BassKernelResults (for profiling)."""
    in_maps = _shard_inputs(inputs["input"], inputs["kernel"])
    return bass_utils.run_bass_kernel_spmd(
        _program(), in_maps, core_ids=list(range(N_CORES)), **kw
    )


def kernel(input, kernel):
    res = run_sharded({"input": input, "kernel": kernel})
    return _unshard_output(res.results)
